# revision 35
# baseline (speedup 1.0000x reference)
"""Trainium2 Bass kernel for nn_AtomUpdateBlock (GemNet AtomUpdateBlock).

Computation (see reference):
    mlp_rbf = rbf @ W_rbf.T            # [E, de]
    x = m * mlp_rbf                    # [E, de]
    x2 = segment_sum(x, id_j, nAtoms)  # [nAtoms, de]
    x = scaled_silu(x2*scale @ W1.T); 2x residual layers; out [nAtoms, da]

Strategy: atom-shard across the 8 cores (12500 atoms each). Host sorts edges
by target atom and greedily packs consecutive atoms into fixed-shape windows
of at most WND=48 atoms and at most KT*128=512 edge slots (~7% edge-slot
padding vs ~10% for fixed atom-width windows). Every window is
exactly KT tiles of 128 edges. On device, each tile computes
x = m * (rbf @ W_rbf.T) via a small matmul + a vector multiply, then
scatter-adds its edges into the window's PSUM accumulator with a one-hot
matmul (onehot[e, w] = (col[e] == w), precomputed host-side, streamed fp8).
Window accumulators live in PSUM in pairs ([128, 128] per pair) and flush to
an SBUF accumulator laid out on a padded atom axis (window w owns columns
[w*64, w*64+64)); the small MLP runs on-chip over that padded axis and the
host drops pad columns when unsharding. Each core owns all edges of its
atoms, so no collectives are needed.

Perf structure (240us vs 399us baseline):
 - the K=16 mlp_rbf matmuls run 4-at-a-time via PE row tiling
   (tile_position=(32q, 0)): operands live on SBUF partition quarters and
   each concurrent pack writes 4 distinct PSUM banks (two groups' buffers
   interleaved), cutting the dominant per-tile LDWEIGHTS+matmul cost ~4x.
 - one-hot streams in fp8e4m3 (0/1 exact) and feeds the scatter matmul
   directly as the fp8 moving operand against fp16 stationary x.
 - 48-atom windows cut the scatter matmul's streamed rows to 3/8 and the
   one-hot bytes to ~40% of the 128-atom-window baseline.
 - m and onehot DMAs are batched 16 tiles per transfer (512KB / 128KB);
   m+rbf+out issue from the Sync sequencer, onehot from Scalar.
 - the x = m*mlp_rbf multiply (DVE, PSUM operand so no 16-bit speedup) is
   the pacing operation; everything else stays off the Vector engine:
   window-pair flushes are Scalar-engine Copy activations, the MLP residual
   combine runs on GpSimd tensor_tensor against memset const tiles (GpSimd
   tensor_scalar is ~7x slower), and W20 @ (s1+s3) is split into two
   accumulating PE matmuls so no explicit add feeds the PE.
 - output is written fp16 (padded axis) and cast/unpacked on host.
"""

import os
import sys
import time
from contextlib import ExitStack

sys.path.insert(0, "/opt/trn_rl_repo")

import numpy as np

NCORES = 8
E = 1_000_000
NATOMS = 100_000
DE = 128
DRBF = 16
P = 128          # edges per tile
WND = 40         # max atoms per window
KT = 3           # tiles per window (fixed; 384 edge slots)
GRP = 8          # tiles per group (one PSUM mlp_rbf batch)
DGRP = 16        # tiles per DMA batch
RBF_CHUNK = 32   # tiles per rbf DMA
MLPW = 512       # atoms per phase-2 (MLP) slice (8 windows)
A_CORE = NATOMS // NCORES          # 12500
PAD_COL = 4096.0                   # one-hot column id that never matches

INV_SCALE_SILU = 1.0 / 0.6
INV_SQRT2 = 2.0 ** -0.5
A_CONST = INV_SCALE_SILU * 0.5           # q * c^2
B_CONST = INV_SCALE_SILU * INV_SQRT2     # q * c

_PROGRAM_CACHE: dict = {}


def _build_program(t_list, epad, ntiles, nwin, atoms_pad):
    import concourse.bacc as bacc
    import concourse.mybir as mybir
    import concourse.tile as tile

    dt = mybir.dt
    op = mybir.AluOpType
    act = mybir.ActivationFunctionType

    nc = bacc.Bacc(
        "TRN2", target_bir_lowering=False, debug=False, num_devices=NCORES
    )

    ngrp = ntiles // GRP

    m_pad = nc.dram_tensor(
        "m_pad", [epad // DGRP, DGRP * DE], dt.float16, kind="ExternalInput"
    ).ap()
    # rbf packed for 4x row-tiled matmuls: tile k of pair j lives on SBUF
    # partition quarter q = k//4 at column block (k%4)*128 of the pair's
    # 512-col block; rows 16..31 of each quarter are zero padding.
    rbf_t = nc.dram_tensor(
        "rbf_t", [P, (ntiles // DGRP) * 4 * P], dt.float16, kind="ExternalInput"
    ).ap()
    oh_in = nc.dram_tensor(
        "oh_in", [epad // DGRP, DGRP * WND], dt.float8e4, kind="ExternalInput"
    ).ap()
    # wrbf replicated on all four partition quarters (rows 32q..32q+15)
    wrbf_in = nc.dram_tensor("wrbf_in", [P, DE], dt.float16, kind="ExternalInput").ap()
    wmlp_in = [
        nc.dram_tensor(f"wmlp{i}_in", [DE, DE], dt.float16, kind="ExternalInput").ap()
        for i in range(5)
    ]
    out = nc.dram_tensor("out", [DE, atoms_pad], dt.float16, kind="ExternalOutput").ap()

    # window bookkeeping: first/last tile of each window
    w_start = []
    w_end = []
    pos = 0
    for t_w in t_list:
        w_start.append(pos)
        w_end.append(pos + t_w - 1)
        pos += t_w
    assert pos == ntiles and ntiles % DGRP == 0 and nwin % 2 == 0
    tile_window = np.repeat(np.arange(nwin), t_list)

    # MLP slices over the padded atom axis
    mlp_slices = []
    a0 = 0
    while a0 < atoms_pad:
        mlp_slices.append((a0, min(a0 + MLPW, atoms_pad)))
        a0 += MLPW
    # slice s is ready once window covering its last atom has flushed; flushes
    # happen per window PAIR, so round up to an odd window index
    mlp_after = {}
    for s, (sa, sb) in enumerate(mlp_slices):
        w = (sb - 1) // WND
        w += (w % 2 == 0)
        mlp_after.setdefault(w, []).append(s)

    with tile.TileContext(nc) as tc, ExitStack() as ctx:
        const_p = ctx.enter_context(tc.tile_pool(name="const_p", bufs=1))
        acc_sb_p = ctx.enter_context(tc.tile_pool(name="acc_sb_p", bufs=1))
        m_p = ctx.enter_context(tc.tile_pool(name="m_p", bufs=4))
        rbfw_p = ctx.enter_context(tc.tile_pool(name="rbfw_p", bufs=3))
        x_p = ctx.enter_context(tc.tile_pool(name="x_p", bufs=10))
        oh_p = ctx.enter_context(tc.tile_pool(name="oh_p", bufs=4))
        mlp_ps_p = ctx.enter_context(tc.tile_pool(name="mlp_ps_p", bufs=2, space="PSUM"))
        acc_ps_p = ctx.enter_context(tc.tile_pool(name="acc_ps_p", bufs=2, space="PSUM"))
        z_ps_p = ctx.enter_context(tc.tile_pool(name="z_ps_p", bufs=2, space="PSUM"))
        s_p = ctx.enter_context(tc.tile_pool(name="s_p", bufs=8))
        o_p = ctx.enter_context(tc.tile_pool(name="o_p", bufs=6))

        # load constants once
        wrbf_sb = const_p.tile([P, DE], dt.float16)
        nc.sync.dma_start(wrbf_sb[:], wrbf_in[:])
        wmlp_sb = []
        for i in range(5):
            wt = const_p.tile([DE, DE], dt.float16, name=f"wmlp_sb{i}")
            nc.sync.dma_start(wt[:], wmlp_in[i][:])
            wmlp_sb.append(wt)
        # broadcast-constant tiles for the GpSimd output combine (GpSimd
        # TENSOR_SCALAR is ~7x slower than TENSOR_TENSOR, so use const tiles)
        bsa_sb = const_p.tile([P, MLPW], dt.float16, name="bsa_sb")
        nc.gpsimd.memset(bsa_sb[:], B_CONST / A_CONST)
        aa_sb = const_p.tile([P, MLPW], dt.float16, name="aa_sb")
        nc.gpsimd.memset(aa_sb[:], A_CONST)

        acc_sb = acc_sb_p.tile([P, atoms_pad], dt.float16)

        # ---- phase 2 (interleaved): MLP slice s once its windows flushed ----
        def emit_mlp(s):
            sa, sb = mlp_slices[s]
            n = sb - sa
            sl = slice(sa, sb)
            z_ps = z_ps_p.tile([P, MLPW], dt.float32, tag="z_ps")
            nc.tensor.matmul(out=z_ps[:, :n], lhsT=wmlp_sb[0][:], rhs=acc_sb[:, sl],
                             start=True, stop=True)
            s1 = s_p.tile([P, MLPW], dt.float16, tag="s1")
            nc.scalar.activation(s1[:, :n], z_ps[:, :n], act.Silu)

            u_ps = z_ps_p.tile([P, MLPW], dt.float32, tag="z_ps")
            nc.tensor.matmul(out=u_ps[:, :n], lhsT=wmlp_sb[1][:], rhs=s1[:, :n],
                             start=True, stop=True)
            s2 = s_p.tile([P, MLPW], dt.float16, tag="s2")
            nc.scalar.activation(s2[:, :n], u_ps[:, :n], act.Silu)

            u2_ps = z_ps_p.tile([P, MLPW], dt.float32, tag="z_ps")
            nc.tensor.matmul(out=u2_ps[:, :n], lhsT=wmlp_sb[2][:], rhs=s2[:, :n],
                             start=True, stop=True)
            s3 = s_p.tile([P, MLPW], dt.float16, tag="s3")
            nc.scalar.activation(s3[:, :n], u2_ps[:, :n], act.Silu)

            # W20 @ (s1 + s3) as two accumulating matmuls - no explicit add
            u3_ps = z_ps_p.tile([P, MLPW], dt.float32, tag="z_ps")
            nc.tensor.matmul(out=u3_ps[:, :n], lhsT=wmlp_sb[3][:], rhs=s1[:, :n],
                             start=True, stop=False)
            nc.tensor.matmul(out=u3_ps[:, :n], lhsT=wmlp_sb[3][:], rhs=s3[:, :n],
                             start=False, stop=True)
            s4 = s_p.tile([P, MLPW], dt.float16, tag="s2")
            nc.scalar.activation(s4[:, :n], u3_ps[:, :n], act.Silu)

            u4_ps = z_ps_p.tile([P, MLPW], dt.float32, tag="z_ps")
            nc.tensor.matmul(out=u4_ps[:, :n], lhsT=wmlp_sb[4][:], rhs=s4[:, :n],
                             start=True, stop=True)
            s5 = s_p.tile([P, MLPW], dt.float16, tag="s5")
            nc.scalar.activation(s5[:, :n], u4_ps[:, :n], act.Silu)

            # out = a*(s1+s3) + b*s5 on GpSimd (all-SBUF fp16), fp16 out
            q1 = o_p.tile([P, MLPW], dt.float16, tag="q1")
            nc.gpsimd.tensor_tensor(out=q1[:, :n], in0=s5[:, :n],
                                    in1=bsa_sb[:, :n], op=op.mult)
            q2 = o_p.tile([P, MLPW], dt.float16, tag="q2")
            nc.gpsimd.tensor_tensor(out=q2[:, :n], in0=s3[:, :n], in1=q1[:, :n],
                                    op=op.add)
            q3 = o_p.tile([P, MLPW], dt.float16, tag="q3")
            nc.gpsimd.tensor_tensor(out=q3[:, :n], in0=s1[:, :n], in1=q2[:, :n],
                                    op=op.add)
            ot = o_p.tile([P, MLPW], dt.float16, tag="ot")
            nc.gpsimd.tensor_tensor(out=ot[:, :n], in0=q3[:, :n],
                                    in1=aa_sb[:, :n], op=op.mult)
            nc.sync.dma_start(out[:, sl], ot[:, :n])

        # ---- phase 1: edge stream -> segment sums ----
        rbf_chunks = []
        acc_ps = None
        xt_hist = {}
        oh_hist = {}

        def emit_front_pair(j):
            # j indexes a pair of groups (16 tiles = one DMA batch)
            if j * DGRP % RBF_CHUNK == 0:
                c0 = j * DGRP  # first tile of chunk
                rbfc = rbfw_p.tile(
                    [P, (RBF_CHUNK // DGRP) * 4 * P], dt.float16, tag="rbfc"
                )
                nc.sync.dma_start(
                    rbfc[:],
                    rbf_t[:, (c0 // DGRP) * 4 * P : (c0 // DGRP + 2) * 4 * P],
                )
                rbf_chunks.append((c0, rbfc))

            m2 = m_p.tile([P, DGRP * DE], dt.float16, tag="m2")
            nc.sync.dma_start(m2[:], m_pad[j * P : (j + 1) * P, :])
            oh2 = oh_p.tile([P, DGRP * WND], dt.float8e4, tag="oh2")
            nc.scalar.dma_start(oh2[:], oh_in[j * P : (j + 1) * P, :])
            for h in range(2):
                g = 2 * j + h
                oh_hist[g] = oh2[:, h * GRP * WND : (h + 1) * GRP * WND]

            c0, rbfc = rbf_chunks[-1]
            base = (j - c0 // DGRP) * 4 * P
            psA = mlp_ps_p.tile([P, GRP * DE], dt.float32, tag="mlp_ps")
            psB = mlp_ps_p.tile([P, GRP * DE], dt.float32, tag="mlp_ps")
            # 4 packs of 4 concurrent row-tiled (K=16) matmuls; each pack's
            # outputs land in 4 distinct PSUM banks (A0, A1, B0, B1)
            for p in range(4):
                lcol = base + p * P
                for q, (ps, sl) in enumerate(
                    [(psA, p), (psA, p + 4), (psB, p), (psB, p + 4)]
                ):
                    nc.tensor.matmul(
                        out=ps[:, sl * DE : (sl + 1) * DE],
                        lhsT=rbfc[32 * q : 32 * q + DRBF, lcol : lcol + P],
                        rhs=wrbf_sb[32 * q : 32 * q + DRBF, :],
                        start=True,
                        stop=True,
                        skip_group_check=True,
                        tile_position=(32 * q, 0),
                    )
            for h, ps in ((0, psA), (1, psB)):
                g = 2 * j + h
                xt4 = x_p.tile([P, GRP * DE], dt.float16, tag="xt4")
                nc.vector.tensor_tensor(
                    out=xt4[:],
                    in0=m2[:, h * GRP * DE : (h + 1) * GRP * DE],
                    in1=ps[:],
                    op=op.mult,
                )
                xt_hist[g] = xt4

        def emit_back(g):
            nonlocal acc_ps
            xt4 = xt_hist.pop(g)
            oh4 = oh_hist.pop(g)
            for i in range(GRP):
                gt = g * GRP + i
                w = tile_window[gt]
                qoff = (w % 2) * WND  # window pairs share one [P, 2*WND] PSUM tile
                if w % 2 == 0 and gt == w_start[w]:
                    acc_ps = acc_ps_p.tile([P, 2 * WND], dt.float32, tag="acc_ps")
                nc.tensor.matmul(
                    out=acc_ps[:, qoff : qoff + WND],
                    lhsT=xt4[:, i * DE : (i + 1) * DE],
                    rhs=oh4[:, i * WND : (i + 1) * WND],
                    start=(gt == w_start[w]),
                    stop=(gt == w_end[w]),
                    skip_group_check=True,
                )
                if w % 2 == 1 and gt == w_end[w]:
                    nc.scalar.copy(
                        out=acc_sb[:, (w - 1) * WND : (w + 1) * WND], in_=acc_ps[:]
                    )
                    for _s in mlp_after.get(w, []):
                        emit_mlp(_s)

        npair = ngrp // 2
        for j in range(npair + 1):
            if j < npair:
                emit_front_pair(j)
            if j > 0:
                emit_back(2 * (j - 1))
                emit_back(2 * (j - 1) + 1)

    nc.compile()
    return nc


def _greedy_windows(deg_core):
    """Pack consecutive atoms into windows of <=WND atoms and <=KT*P edges.
    Returns list of (a0, a1) atom ranges (core-local indices)."""
    cap = KT * P
    wins = []
    a0 = 0
    a_in = 0
    e_in = 0
    for a, dg in enumerate(deg_core):
        if a_in >= WND or e_in + dg > cap:
            wins.append((a0, a))
            a0, a_in, e_in = a, 0, 0
        a_in += 1
        e_in += int(dg)
    wins.append((a0, len(deg_core)))
    return wins


def _prepare(m, rbf, id_j, W_rbf, scale, W1, W_res):
    """Host-side: sort edges by atom, greedily window, pad, shard per core."""
    import concourse.mybir as mybir

    f8 = mybir.dt.np(mybir.dt.float8e4)

    id_j = np.ascontiguousarray(np.asarray(id_j).astype(np.int64))
    perm = np.argsort(id_j, kind="stable")
    ids_sorted = id_j[perm]
    deg = np.bincount(id_j, minlength=NATOMS)

    wins_per_core = []
    for c in range(NCORES):
        wins_per_core.append(_greedy_windows(deg[c * A_CORE : (c + 1) * A_CORE]))
    nwin = max(len(w) for w in wins_per_core)
    nwin += nwin % 2  # window pairs share a PSUM tile
    atoms_pad = nwin * WND

    t_list = np.full(nwin, KT, dtype=np.int64)
    rem = (-int(t_list.sum())) % RBF_CHUNK
    t_list[-1] += rem
    ntiles = int(t_list.sum())
    epad = ntiles * P
    slot0 = np.concatenate([[0], np.cumsum(t_list) * P])  # window -> first slot

    gidx = np.zeros((NCORES, epad), dtype=np.int64)
    cols = np.full((NCORES, epad), PAD_COL, dtype=np.float32)
    unpack = np.zeros((NCORES, A_CORE), dtype=np.int64)  # atom -> padded column
    for c in range(NCORES):
        wins = wins_per_core[c]
        # edge range of each window via searchsorted on global atom ids
        a_bounds = np.array(
            [c * A_CORE + a0 for a0, _ in wins] + [c * A_CORE + wins[-1][1]],
            dtype=np.int64,
        )
        e_bounds = np.searchsorted(ids_sorted, a_bounds)
        for w, (a0, a1) in enumerate(wins):
            s0, e0 = e_bounds[w], e_bounds[w + 1]
            n = e0 - s0
            pos = slot0[w]
            assert n <= t_list[w] * P
            gidx[c, pos : pos + n] = perm[s0:e0]
            if n < t_list[w] * P:
                gidx[c, pos + n : pos + t_list[w] * P] = perm[s0] if n > 0 else 0
            cols[c, pos : pos + n] = ids_sorted[s0:e0] - (c * A_CORE + a0)
            unpack[c, a0:a1] = w * WND + np.arange(a1 - a0)

    # constants / weights
    q = INV_SCALE_SILU
    c2 = INV_SQRT2
    scale = float(np.asarray(scale))
    # wrbf replicated on all four partition quarters for row-tiled matmuls
    wrbf_np = np.zeros((P, DE), dtype=np.float16)
    for qq in range(4):
        wrbf_np[32 * qq : 32 * qq + DRBF] = W_rbf.T.astype(np.float16)
    wrbf_np = np.ascontiguousarray(wrbf_np)
    wmlp_np = [
        np.ascontiguousarray((W1 * scale).T).astype(np.float16),
        np.ascontiguousarray((W_res[0, 0] * q).T).astype(np.float16),
        np.ascontiguousarray((W_res[0, 1] * q).T).astype(np.float16),
        np.ascontiguousarray((W_res[1, 0] * (q * c2)).T).astype(np.float16),
        np.ascontiguousarray((W_res[1, 1] * q).T).astype(np.float16),
    ]

    in_maps = []
    ndg = epad // (DGRP * P)
    for c in range(NCORES):
        g = gidx[c]
        m_pad = np.ascontiguousarray(
            m[g].astype(np.float16).reshape(ndg, DGRP, P, DE)
            .transpose(0, 2, 1, 3).reshape(ndg * P, DGRP * DE)
        )  # row j*128+p = 16 tiles' row p, contiguous 4KB
        # rbf packed for row-tiled matmuls: [128, npair*512], tile k of pair j
        # -> partition quarter k//4, column block j*512 + (k%4)*128
        rt = rbf[g].astype(np.float16).reshape(ndg, DGRP, P, DRBF)  # [j,k,e,r]
        rbf_t = np.zeros((P, ndg * 4 * P), dtype=np.float16)
        r4 = rbf_t.reshape(P, ndg, 4, P)
        for k in range(DGRP):
            qq, pc = k // 4, k % 4
            r4[32 * qq : 32 * qq + DRBF, :, pc, :] = rt[:, k].transpose(2, 0, 1)
        rbf_t = np.ascontiguousarray(rbf_t)
        oh = (
            cols[c].astype(np.int32)[:, None] == np.arange(WND, dtype=np.int32)[None, :]
        ).astype(f8).reshape(ndg, DGRP, P, WND)
        oh = np.ascontiguousarray(
            oh.transpose(0, 2, 1, 3).reshape(ndg * P, DGRP * WND)
        )
        im = {
            "m_pad": m_pad,
            "rbf_t": rbf_t,
            "oh_in": oh,
            "wrbf_in": wrbf_np,
        }
        for i in range(5):
            im[f"wmlp{i}_in"] = wmlp_np[i]
        in_maps.append(im)

    return (tuple(t_list.tolist()), epad, ntiles, nwin, atoms_pad), in_maps, unpack


def _run(inputs, trace=False):
    from concourse.bass_utils import run_bass_kernel_spmd

    nAtoms = int(np.asarray(inputs["nAtoms"]))
    assert nAtoms == NATOMS, f"kernel hardcoded for nAtoms={NATOMS}, got {nAtoms}"
    m = np.asarray(inputs["m"], dtype=np.float32)
    assert m.shape == (E, DE), m.shape

    key, in_maps, unpack = _prepare(
        m,
        np.asarray(inputs["rbf"], dtype=np.float32),
        inputs["id_j"],
        np.asarray(inputs["W_rbf"], dtype=np.float32),
        inputs["scale"],
        np.asarray(inputs["W1"], dtype=np.float32),
        np.asarray(inputs["W_res"], dtype=np.float32),
    )

    if key not in _PROGRAM_CACHE:
        _PROGRAM_CACHE.clear()
        _PROGRAM_CACHE[key] = _build_program(*key)
    nc = _PROGRAM_CACHE[key]

    res = run_bass_kernel_spmd(
        nc, in_maps, core_ids=list(range(NCORES)), trace=trace
    )
    out_full = np.empty((NATOMS, DE), dtype=np.float32)
    for c in range(NCORES):
        oc = res.results[c]["out"].astype(np.float32)  # [DE, atoms_pad]
        out_full[c * A_CORE : (c + 1) * A_CORE] = oc[:, unpack[c]].T
    return out_full, res.exec_time_ns


def kernel(**inputs) -> np.ndarray:
    out, _ = _run(inputs, trace=False)
    return out
